# revision 1
# baseline (speedup 1.0000x reference)
"""Trainium2 Bass kernel: per-channel circular conv via DFT matmuls, summed
over channels (sparse PSF kernel), 8-core channel-sharded SPMD.

out[b] = irfft2( sum_c rfft2(x[b,c]) * rfft2(scatter(relu(vk), idx)[c]) )

Sharding: each core owns 4 of 32 channels (forward FFTs + pointwise
multiply-accumulate), ReduceScatter(add) over batch gives core b the summed
spectrum of batch b, which it inverse-transforms. All FFTs are dense DFT
matmuls in float32r (full PE rate at moving-dim >= 256, even N required).

Spectra are kept transposed ("T-form", [q (0..256) x j (0..511)]) with the
m>256 half stored conjugated at its natural compute position ("P-form") so
no data reversal is ever needed - all permutations/conjugations/signs are
absorbed into host-precomputed constant matrices, including the inverse.
"""
import numpy as np

N = 512
NQ = 257
NE = 258          # even-padded 257 (fp32r matmul needs even moving dim)
NB = 8            # batches (one per core after reduce-scatter)
CL = 4            # channels per core
NC_TOT = 32
NCORES = 8
TH = 2 * np.pi / N
PB = 2 * 2 * N * 128 + 2 * N   # per-batch rs payload: 2 qchunks x 2 planes + nyq r/i

_CACHE = {}


def _consts():
    r = np.arange(N)
    m = np.arange(NQ)
    ang1 = TH * np.outer(r, m)
    FrT = np.zeros((N, NE), np.float32)
    FiT = np.zeros((N, NE), np.float32)
    FrT[:, :NQ] = np.cos(ang1)
    FiT[:, :NQ] = -np.sin(ang1)
    q = np.arange(256)
    ang2 = TH * np.outer(r, q)
    GrT = np.cos(ang2).astype(np.float32)
    GiT = (-np.sin(ang2)).astype(np.float32)
    GnT = -GiT
    altT = ((-1.0) ** r).astype(np.float32).reshape(N, 1)
    w = np.full(NQ, 2.0)
    w[0] = 1.0
    w[256] = 1.0
    angA = TH * np.outer(np.arange(NQ), r)
    Acos = (w[:, None] * np.cos(angA)).astype(np.float32)
    Asin = (w[:, None] * np.sin(angA)).astype(np.float32)
    Ansin = -Asin
    j = np.arange(N)
    angB = TH * np.outer(j, r)
    sgn = np.ones((N, N))
    sgn[257:, :] = ((-1.0) ** r)[None, :]
    Bcos_t = (np.cos(angB) * sgn / (N * N)).astype(np.float32)
    Bsin_t = (-np.sin(angB) * sgn / (N * N)).astype(np.float32)

    def bpack(Bm):
        out = np.zeros((640, N), np.float32)
        out[0:128] = Bm[0:128]
        out[128:256] = Bm[128:256]
        out[256:256 + 127] = Bm[257:384]
        out[384:512] = Bm[384:512]
        out[512:513] = Bm[256:257]
        return out
    Bcos = bpack(Bcos_t)
    Bsin = bpack(Bsin_t)
    ones4 = np.ones((CL, 1), np.float32)
    return dict(FrT=FrT, FiT=FiT, GrT=GrT, GiT=GiT, GnT=GnT, altT=altT,
                Acos=Acos, Asin=Asin, Ansin=Ansin, Bcos=Bcos, Bsin=Bsin,
                ones4=ones4)


CONST_SHAPES = dict(FrT=(N, NE), FiT=(N, NE), GrT=(N, 256), GiT=(N, 256),
                    GnT=(N, 256), altT=(N, 1), Acos=(NQ, N), Asin=(NQ, N),
                    Ansin=(NQ, N), Bcos=(640, N), Bsin=(640, N), ones4=(CL, 1))


def _build_nc(repeat=1):
    import concourse.bacc as bacc
    import concourse.mybir as mybir
    import concourse.tile as tile

    f32 = mybir.dt.float32
    f32r = mybir.dt.float32r
    mult = mybir.AluOpType.mult
    add = mybir.AluOpType.add
    sub = mybir.AluOpType.subtract

    nc = bacc.Bacc("TRN2", target_bir_lowering=False, debug=False,
                   enable_asserts=False, num_devices=NCORES)
    xs_in = nc.dram_tensor("xs", [NB * CL, N, N], f32r, kind="ExternalInput")
    kd_in = nc.dram_tensor("kd", [CL, N, N], f32r, kind="ExternalInput")
    cins = {nm: nc.dram_tensor(nm, list(sh), f32r, kind="ExternalInput")
            for nm, sh in CONST_SHAPES.items()}
    y_out = nc.dram_tensor("y", [N, N], f32, kind="ExternalOutput")

    with tile.TileContext(nc) as tc:
        with tc.tile_pool(name="consts", bufs=1) as cp, \
             tc.tile_pool(name="kf", bufs=1) as kp, \
             tc.tile_pool(name="xio", bufs=8) as xp, \
             tc.tile_pool(name="crt", bufs=8) as crp, \
             tc.tile_pool(name="acc", bufs=2) as ap, \
             tc.tile_pool(name="tmp", bufs=4) as tp, \
             tc.tile_pool(name="tmp2", bufs=2) as tp2, \
             tc.tile_pool(name="inv", bufs=1) as ivp, \
             tc.tile_pool(name="psA", bufs=2, space="PSUM") as psA, \
             tc.tile_pool(name="psB", bufs=4, space="PSUM") as psB, \
             tc.tile_pool(name="psN", bufs=2, space="PSUM") as psN, \
             tc.tile_pool(name="dram", bufs=1, space="DRAM") as dp:

            # ---- load constants (chunked along partition) ----
            def load_const(nm, rows, cols):
                ts = []
                nch = (rows + 127) // 128
                for k in range(nch):
                    p = min(128, rows - k * 128)
                    t = cp.tile([p, cols], f32r, name=f"{nm}{k}", tag=f"{nm}{k}")
                    nc.sync.dma_start(t[:], cins[nm][k * 128:k * 128 + p, :])
                    ts.append(t)
                return ts

            Fr = load_const("FrT", N, NE)
            Fi = load_const("FiT", N, NE)
            Gr = load_const("GrT", N, 256)
            Gi = load_const("GiT", N, 256)
            Gn = load_const("GnT", N, 256)
            alt = load_const("altT", N, 1)
            Ac = load_const("Acos", NQ, N)   # chunks: 128,128,1
            As = load_const("Asin", NQ, N)
            An = load_const("Ansin", NQ, N)
            Bc = load_const("Bcos", 640, N)
            Bs = load_const("Bsin", 640, N)
            o4 = load_const("ones4", CL, 1)

            # ---- persistent Kf storage ----
            kfr = [[kp.tile([128, N], f32, name=f"kfr{c}_{qc}", tag=f"kfr{c}_{qc}") for qc in range(2)]
                   for c in range(CL)]
            kfi = [[kp.tile([128, N], f32, name=f"kfi{c}_{qc}", tag=f"kfi{c}_{qc}") for qc in range(2)]
                   for c in range(CL)]
            kfnr = kp.tile([CL, NE], f32, name="kfnr", tag="kfnr")   # nyq strips packed by channel
            kfni = kp.tile([CL, NE], f32, name="kfni", tag="kfni")

            psum_free = {}

            def forward_image(src, mode, c, acc=None, xnyq=None):
                """src: DRAM AP [N, N]. mode 'k' fills kf tiles for channel c;
                mode 'x' pointwise-accumulates vs kf into acc tiles."""
                xt = []
                for k in range(4):
                    t = xp.tile([128, N], f32r, name="xt", tag="xt")
                    nc.sync.dma_start(t[:], src[k * 128:(k + 1) * 128, :])
                    xt.append(t)
                # stage 1: C^T[n, m] (m in [0,257), col 257 zero)
                crt, cit = [], []
                for n in range(4):
                    pr = psA.tile([128, NE], f32, name="ps1r", tag="ps1")
                    pi = psA.tile([128, NE], f32, name="ps1i", tag="ps1")
                    for k in range(4):
                        lhs = xt[k][:, n * 128:(n + 1) * 128]
                        nc.tensor.matmul(pr[:], lhsT=lhs, rhs=Fr[k][:],
                                         start=(k == 0), stop=(k == 3))
                        nc.tensor.matmul(pi[:], lhsT=lhs, rhs=Fi[k][:],
                                         start=(k == 0), stop=(k == 3))
                    cr = crp.tile([128, NE], f32r, name="cr", tag="cr")
                    ci = crp.tile([128, NE], f32r, name="ci", tag="ci")
                    nc.scalar.copy(cr[:], pr[:])
                    nc.scalar.copy(ci[:], pi[:])
                    crt.append(cr)
                    cit.append(ci)

                # stage 2 per q-chunk: 4 accumulation groups
                for qc in range(2):
                    qs = slice(qc * 128, (qc + 1) * 128)
                    ra = psB.tile([128, NE], f32, name="ra", tag="ps2")
                    ia = psB.tile([128, NE], f32, name="ia", tag="ps2")
                    rb = psB.tile([128, 256], f32, name="rb", tag="ps2")
                    ib = psB.tile([128, 256], f32, name="ib", tag="ps2")
                    for k in range(4):
                        st, sp = (k == 0), (k == 3)
                        a_s = slice(0, NE)
                        b_s = slice(0, 256)
                        nc.tensor.matmul(ra[:], lhsT=Gr[k][:, qs], rhs=crt[k][:, a_s], start=st, stop=False)
                        nc.tensor.matmul(ra[:], lhsT=Gn[k][:, qs], rhs=cit[k][:, a_s], start=False, stop=sp)
                        nc.tensor.matmul(ia[:], lhsT=Gi[k][:, qs], rhs=crt[k][:, a_s], start=st, stop=False)
                        nc.tensor.matmul(ia[:], lhsT=Gr[k][:, qs], rhs=cit[k][:, a_s], start=False, stop=sp)
                        nc.tensor.matmul(rb[:], lhsT=Gr[k][:, qs], rhs=crt[k][:, b_s], start=st, stop=False)
                        nc.tensor.matmul(rb[:], lhsT=Gi[k][:, qs], rhs=cit[k][:, b_s], start=False, stop=sp)
                        nc.tensor.matmul(ib[:], lhsT=Gr[k][:, qs], rhs=cit[k][:, b_s], start=st, stop=False)
                        nc.tensor.matmul(ib[:], lhsT=Gn[k][:, qs], rhs=crt[k][:, b_s], start=False, stop=sp)
                    if mode == "k":
                        nc.scalar.copy(kfr[c][qc][:, 0:NQ], ra[:, 0:NQ])
                        nc.scalar.copy(kfi[c][qc][:, 0:NQ], ia[:, 0:NQ])
                        nc.scalar.copy(kfr[c][qc][:, NQ:N], rb[:, 1:256])
                        nc.scalar.copy(kfi[c][qc][:, NQ:N], ib[:, 1:256])
                    else:
                        ar, ai = acc[qc]
                        # region a: cols [0:257] ; region b: cols [257:512]
                        for (px, cols_p, cols_a) in ((ra, slice(0, NQ), slice(0, NQ)),
                                                     (rb, slice(1, 256), slice(NQ, N))):
                            pxi = ia if px is ra else ib
                            kr = kfr[c][qc][:, cols_a]
                            ki = kfi[c][qc][:, cols_a]
                            t1 = tp.tile([128, NQ], f32, name="t1", tag="t1")
                            t2 = tp.tile([128, NQ], f32, name="t2", tag="t2")
                            w = cols_p.stop - cols_p.start
                            nc.vector.scalar_tensor_tensor(t1[:, :w], px[:, cols_p], 1.0, kr, op0=mult, op1=mult)
                            nc.vector.scalar_tensor_tensor(t2[:, :w], pxi[:, cols_p], 1.0, ki, op0=mult, op1=mult)
                            if c == 0:
                                nc.vector.scalar_tensor_tensor(ar[:, cols_a], t1[:, :w], 1.0, t2[:, :w], op0=mult, op1=sub)
                            else:
                                u = tp.tile([128, NQ], f32, name="u", tag="u")
                                nc.vector.scalar_tensor_tensor(u[:, :w], t1[:, :w], 1.0, t2[:, :w], op0=mult, op1=sub)
                                nc.vector.scalar_tensor_tensor(ar[:, cols_a], ar[:, cols_a], 0.0, u[:, :w], op0=add, op1=add)
                            nc.vector.scalar_tensor_tensor(t1[:, :w], px[:, cols_p], 1.0, ki, op0=mult, op1=mult)
                            nc.vector.scalar_tensor_tensor(t2[:, :w], pxi[:, cols_p], 1.0, kr, op0=mult, op1=mult)
                            if c == 0:
                                nc.vector.scalar_tensor_tensor(ai[:, cols_a], t1[:, :w], 1.0, t2[:, :w], op0=mult, op1=add)
                            else:
                                u = tp.tile([128, NQ], f32, name="u", tag="u")
                                nc.vector.scalar_tensor_tensor(u[:, :w], t1[:, :w], 1.0, t2[:, :w], op0=mult, op1=add)
                                nc.vector.scalar_tensor_tensor(ai[:, cols_a], ai[:, cols_a], 0.0, u[:, :w], op0=add, op1=add)

                # nyquist strip q=256 (j in [0,258))
                nr = psN.tile([1, NE], f32, name="nr", tag="psn")
                ni = psN.tile([1, NE], f32, name="ni", tag="psn")
                for k in range(4):
                    st, sp = (k == 0), (k == 3)
                    nc.tensor.matmul(nr[:], lhsT=alt[k][:], rhs=crt[k][:], start=st, stop=sp)
                    nc.tensor.matmul(ni[:], lhsT=alt[k][:], rhs=cit[k][:], start=st, stop=sp)
                sr = tp2.tile([1, NE], f32, name="nstr", tag="nstr")
                si = tp2.tile([1, NE], f32, name="nsti", tag="nstr")
                nc.scalar.copy(sr[:], nr[:])
                nc.scalar.copy(si[:], ni[:])
                dst = (kfnr, kfni) if mode == "k" else xnyq
                nc.sync.dma_start(dst[0][c:c + 1, :], sr[:])
                nc.sync.dma_start(dst[1][c:c + 1, :], si[:])

            rs_in = dp.tile([NCORES, PB], f32, name="rs_in")
            rs_out = dp.tile([PB], f32, name="rs_out")
            for _rep in range(repeat):
                # ---- phase 1: kernel channels ----
                for c in range(CL):
                    forward_image(kd_in[c], "k", c)

                # ---- phase 2: batches ----
                for b in range(NB):
                    acc = [(ap.tile([128, N], f32, name="accr", tag=f"accr{qc}"),
                            ap.tile([128, N], f32, name="acci", tag=f"acci{qc}"))
                           for qc in range(2)]
                    xnr = ap.tile([CL, NE], f32, name="xnr", tag="xnr")
                    xni = ap.tile([CL, NE], f32, name="xni", tag="xni")
                    for c in range(CL):
                        forward_image(xs_in[b * CL + c], "x", c, acc=acc, xnyq=(xnr, xni))
                    # nyq pointwise (per-channel rows) + channel reduce via K=4 matmul
                    t1 = tp2.tile([CL, NE], f32, name="nt1", tag="nt")
                    t2 = tp2.tile([CL, NE], f32, name="nt2", tag="nt")
                    un = tp2.tile([CL, NE], f32r, name="un", tag="un")
                    vn = tp2.tile([CL, NE], f32r, name="vn", tag="un")
                    nc.vector.scalar_tensor_tensor(t1[:], xnr[:], 1.0, kfnr[:], op0=mult, op1=mult)
                    nc.vector.scalar_tensor_tensor(t2[:], xni[:], 1.0, kfni[:], op0=mult, op1=mult)
                    nc.vector.scalar_tensor_tensor(un[:], t1[:], 1.0, t2[:], op0=mult, op1=sub)
                    nc.vector.scalar_tensor_tensor(t1[:], xnr[:], 1.0, kfni[:], op0=mult, op1=mult)
                    nc.vector.scalar_tensor_tensor(t2[:], xni[:], 1.0, kfnr[:], op0=mult, op1=mult)
                    nc.vector.scalar_tensor_tensor(vn[:], t1[:], 1.0, t2[:], op0=mult, op1=add)
                    pnr = psN.tile([1, NE], f32, name="pnr", tag="psn")
                    pni = psN.tile([1, NE], f32, name="pni", tag="psn")
                    nc.tensor.matmul(pnr[:], lhsT=o4[0][:], rhs=un[:], start=True, stop=True)
                    nc.tensor.matmul(pni[:], lhsT=o4[0][:], rhs=vn[:], start=True, stop=True)
                    snr = tp2.tile([1, NE], f32, name="snr", tag="sn")
                    sni = tp2.tile([1, NE], f32, name="sni", tag="sn")
                    nc.scalar.copy(snr[:], pnr[:])
                    nc.scalar.copy(sni[:], pni[:])
                    # ship batch partial to rs_in row b
                    row = rs_in[b]
                    for qc in range(2):
                        ar, ai = acc[qc]
                        base = qc * 2 * 128 * N
                        nc.sync.dma_start(row[base:base + 128 * N].rearrange("(p f) -> p f", p=128), ar[:])
                        nc.sync.dma_start(row[base + 128 * N:base + 2 * 128 * N].rearrange("(p f) -> p f", p=128), ai[:])
                    nyb = 2 * 2 * 128 * N
                    nc.sync.dma_start(row[nyb:nyb + NQ].rearrange("(p f) -> p f", p=1), snr[:, 0:NQ])
                    nc.sync.dma_start(row[nyb + N:nyb + N + NQ].rearrange("(p f) -> p f", p=1), sni[:, 0:NQ])

                # ---- phase 3: reduce-scatter ----
                nc.gpsimd.collective_compute(
                    "ReduceScatter", mybir.AluOpType.add,
                    replica_groups=[list(range(NCORES))],
                    ins=[rs_in[:].opt()], outs=[rs_out[:].opt()],
                )

                # ---- phase 4: inverse on my batch ----
                Tr = [ivp.tile([128, N], f32r, name=f"Tr{qc}", tag=f"Tr{qc}") for qc in range(2)]
                Ti = [ivp.tile([128, N], f32r, name=f"Ti{qc}", tag=f"Ti{qc}") for qc in range(2)]
                tnr = ivp.tile([1, N], f32r, name="tnr", tag="tnr")
                tni = ivp.tile([1, N], f32r, name="tni", tag="tni")
                for qc in range(2):
                    base = qc * 2 * 128 * N
                    nc.gpsimd.dma_start(Tr[qc][:], rs_out[base:base + 128 * N].rearrange("(p f) -> p f", p=128))
                    nc.gpsimd.dma_start(Ti[qc][:], rs_out[base + 128 * N:base + 2 * 128 * N].rearrange("(p f) -> p f", p=128))
                nyb = 2 * 2 * 128 * N
                nc.gpsimd.dma_start(tnr[:, 0:NQ], rs_out[nyb:nyb + NQ].rearrange("(p f) -> p f", p=1))
                nc.gpsimd.dma_start(tni[:, 0:NQ], rs_out[nyb + N:nyb + N + NQ].rearrange("(p f) -> p f", p=1))
                # nyq fixup: T[256, 257:512] = T[256, 1:256]
                nc.vector.scalar_tensor_tensor(tnr[:, NQ:N], tnr[:, 1:256], 0.0, tnr[:, 1:256], op0=mult, op1=add)
                nc.vector.scalar_tensor_tensor(tni[:, NQ:N], tni[:, 1:256], 0.0, tni[:, 1:256], op0=mult, op1=add)

                # inv stage 1: R[j, n] per j-chunk; slices [0:128],[128:256],[257:385],[385:512], plus j=256 strip
                jsl = [(slice(0, 128), 128, True), (slice(128, 256), 128, True),
                       (slice(257, 384), 127, False), (slice(384, 512), 128, False)]
                Rr, Ri = [], []
                for (js, mw, plus) in jsl:
                    prr = psB.tile([mw, N], f32, name="prr", tag="ps2")
                    pri = psB.tile([mw, N], f32, name="pri", tag="ps2")
                    for qk in range(3):       # q chunks: 128,128,1(nyq strip)
                        st, sp = (qk == 0), (qk == 2)
                        if qk < 2:
                            lr, li = Tr[qk][:, js], Ti[qk][:, js]
                        else:
                            lr, li = tnr[:, js], tni[:, js]
                        # Rr = Tr.Acos -/+ Ti.Asin ; sign folded via const choice
                        nc.tensor.matmul(prr[:], lhsT=lr, rhs=Ac[qk][:], start=st, stop=False)
                        nc.tensor.matmul(prr[:], lhsT=li, rhs=(An if plus else As)[qk][:], start=False, stop=sp)
                        # Ri = Tr.(+/-Asin) + Ti.Acos
                        nc.tensor.matmul(pri[:], lhsT=lr, rhs=(As if plus else An)[qk][:], start=st, stop=False)
                        nc.tensor.matmul(pri[:], lhsT=li, rhs=Ac[qk][:], start=False, stop=sp)
                    rr = ivp.tile([mw, N], f32r, name="rr", tag=f"rr{js.start}")
                    ri = ivp.tile([mw, N], f32r, name="ri", tag=f"ri{js.start}")
                    nc.scalar.copy(rr[:], prr[:])
                    nc.scalar.copy(ri[:], pri[:])
                    Rr.append(rr)
                    Ri.append(ri)
                # j=256 column strip (uses '+' signs)
                p6r = psN.tile([1, N], f32, name="p6r", tag="psn")
                p6i = psN.tile([1, N], f32, name="p6i", tag="psn")
                for qk in range(3):
                    st, sp = (qk == 0), (qk == 2)
                    if qk < 2:
                        lr, li = Tr[qk][:, 256:257], Ti[qk][:, 256:257]
                    else:
                        lr, li = tnr[:, 256:257], tni[:, 256:257]
                    nc.tensor.matmul(p6r[:], lhsT=lr, rhs=Ac[qk][:], start=st, stop=False)
                    nc.tensor.matmul(p6r[:], lhsT=li, rhs=An[qk][:], start=False, stop=sp)
                    nc.tensor.matmul(p6i[:], lhsT=lr, rhs=As[qk][:], start=st, stop=False)
                    nc.tensor.matmul(p6i[:], lhsT=li, rhs=Ac[qk][:], start=False, stop=sp)
                r6r = ivp.tile([1, N], f32r, name="r6r", tag="r6r")
                r6i = ivp.tile([1, N], f32r, name="r6i", tag="r6i")
                nc.scalar.copy(r6r[:], p6r[:])
                nc.scalar.copy(r6i[:], p6i[:])

                # inv stage 2: y[r, n] = sum_j Bcos[j,r].Rr[j,n] + Bsin[j,r].Ri[j,n]
                jrows = [slice(0, 128), slice(128, 256), slice(257, 384), slice(384, 512)]
                for rc in range(4):
                    rs = slice(rc * 128, (rc + 1) * 128)
                    py = psB.tile([128, N], f32, name="py", tag="ps2")
                    kws = [128, 128, 127, 128]
                    for ji in range(5):
                        st = (ji == 0)
                        if ji < 4:
                            kw = kws[ji]
                            nc.tensor.matmul(py[:], lhsT=Bc[ji][0:kw, rs], rhs=Rr[ji][:], start=st, stop=False)
                            nc.tensor.matmul(py[:], lhsT=Bs[ji][0:kw, rs], rhs=Ri[ji][:], start=False, stop=False)
                        else:
                            nc.tensor.matmul(py[:], lhsT=Bc[4][0:1, rs], rhs=r6r[:], start=False, stop=False)
                            nc.tensor.matmul(py[:], lhsT=Bs[4][0:1, rs], rhs=r6i[:], start=False, stop=True)
                    yt = tp2.tile([128, N], f32, name="yt", tag="yt")
                    nc.scalar.copy(yt[:], py[:])
                    nc.sync.dma_start(y_out[rs, :], yt[:])

    nc.compile()
    return nc


def _get_nc(repeat=1):
    key = f"nc{repeat}"
    if key not in _CACHE:
        _CACHE[key] = _build_nc(repeat)
    return _CACHE[key]


def kernel(x, kernel, variable_kernel, loc_idx, _repeat=1):
    from concourse.bass_utils import run_bass_kernel_spmd

    x = np.asarray(x)
    vk = np.asarray(variable_kernel)
    idx = np.asarray(loc_idx)
    # host scatter of relu'd values into the dense PSF (data movement only)
    kflat = np.asarray(kernel)[0].reshape(-1).copy()
    kflat[idx] = np.maximum(vk, 0.0).astype(np.float32)
    kd_all = kflat.reshape(NC_TOT, N, N)

    C = _consts()
    nc = _get_nc(_repeat)
    in_maps = []
    for core in range(NCORES):
        c0 = core * CL
        m = {nm: C[nm] for nm in CONST_SHAPES}
        m["xs"] = np.ascontiguousarray(
            x[:, c0:c0 + CL].reshape(NB * CL, N, N)).astype(np.float32)
        m["kd"] = np.ascontiguousarray(kd_all[c0:c0 + CL]).astype(np.float32)
        in_maps.append(m)
    res = run_bass_kernel_spmd(nc, in_maps, core_ids=list(range(NCORES)))
    out = np.stack([res.results[b]["y"] for b in range(NB)], axis=0)
    return out.astype(np.float32)



# revision 34
# speedup vs baseline: 3885.6087x; 3885.6087x over previous
"""Trainium2 Bass kernel: per-channel circular conv via DFT matmuls, summed
over channels (sparse PSF kernel), 8-core channel-sharded SPMD.

out[b] = irfft2( sum_c rfft2(x[b,c]) * rfft2(scatter(relu(vk), idx)[c]) )

Sharding: each core owns 4 of 32 channels (forward FFTs + pointwise
multiply-accumulate), ReduceScatter(add) over batch gives core b the summed
spectrum of batch b, which it inverse-transforms. All FFTs are dense DFT
matmuls in float32r (full PE rate at moving-dim >= 256, even N required).

Spectra are kept transposed ("T-form", [q (0..256) x j (0..511)]) with the
m>256 half stored conjugated at its natural compute position ("P-form") so
no data reversal is ever needed - all permutations/conjugations/signs are
absorbed into host-precomputed constant matrices, including the inverse.
"""
import numpy as np

N = 512
NQ = 257
NE = 258          # even-padded 257 (fp32r matmul needs even moving dim)
W = 514           # packed plane width: [m 0..257 | conj-m 0..255]
NB = 8            # batches (one per core after reduce-scatter)
CL = 4            # channels per core
NC_TOT = 32
NCORES = 8
TH = 2 * np.pi / N
PB = 4 * 128 * W + 2 * NE  # per-batch rs payload (bf16): 4 planes + nyq r|i

RS_F32 = True
_CACHE = {}


def _consts():
    if "consts" in _CACHE:
        return _CACHE["consts"]
    r = np.arange(N)
    m = np.arange(NQ)
    ang1 = TH * np.outer(r, m)
    FrT = np.zeros((N, NE), np.float32)
    FiT = np.zeros((N, NE), np.float32)
    FrT[:, :NQ] = np.cos(ang1)
    FiT[:, :NQ] = -np.sin(ang1)
    n2 = np.arange(256)
    q2 = np.arange(128)
    ang_e = 2 * np.pi * np.outer(n2, q2) / 256.0
    ang_o = 2 * np.pi * np.outer(n2, 2 * q2 + 1) / 512.0
    E2c = np.cos(ang_e).astype(np.float32)
    E2p = np.sin(ang_e).astype(np.float32)
    E2m = -E2p
    O2c = np.cos(ang_o).astype(np.float32)
    O2p = np.sin(ang_o).astype(np.float32)
    O2m = -O2p
    altT = ((-1.0) ** r).astype(np.float32).reshape(N, 1)
    w = np.full(NQ, 2.0)
    w[0] = 1.0
    w[256] = 1.0
    angA = TH * np.outer(np.arange(NQ), r)
    # rows permuted to DIF order: even q, odd q, then q=256 strip
    qperm = np.concatenate([np.arange(0, 256, 2), np.arange(1, 256, 2), [256]])
    Acos = (w[:, None] * np.cos(angA)).astype(np.float32)[qperm]
    Asin = (w[:, None] * np.sin(angA)).astype(np.float32)[qperm]
    Ansin = -Asin
    j = np.arange(N)
    angB = TH * np.outer(j, r)
    sgn = np.ones((N, N))
    sgn[257:, :] = ((-1.0) ** r)[None, :]
    Bcos_t = (np.cos(angB) * sgn / (N * N)).astype(np.float32)
    Bsin_t = (-np.sin(angB) * sgn / (N * N)).astype(np.float32)

    def bpack(Bm):
        out = np.zeros((640, N), np.float32)
        out[0:128] = Bm[0:128]
        out[128:256] = Bm[128:256]
        out[256:256 + 127] = Bm[257:384]
        out[384:512] = Bm[384:512]
        out[512:513] = Bm[256:257]
        return out
    Bcos = bpack(Bcos_t)
    Bsin = bpack(Bsin_t)
    ones4 = np.ones((CL, 1), np.float32)
    out = dict(FrT=FrT, FiT=FiT, E2c=E2c, E2p=E2p, E2m=E2m,
               O2c=O2c, O2p=O2p, O2m=O2m, altT=altT,
               Acos=Acos, Asin=Asin, Ansin=Ansin, Bcos=Bcos, Bsin=Bsin,
               ones4=ones4)
    _CACHE["consts"] = out
    return out


CONST_SHAPES = dict(FrT=(N, NE), FiT=(N, NE), E2c=(256, 128), E2p=(256, 128),
                    E2m=(256, 128), O2c=(256, 128), O2p=(256, 128),
                    O2m=(256, 128), altT=(N, 1), Acos=(NQ, N), Asin=(NQ, N),
                    Ansin=(NQ, N), Bcos=(640, N), Bsin=(640, N), ones4=(CL, 1))


def _build_nc(repeat=1):
    import concourse.bacc as bacc
    import concourse.mybir as mybir
    import concourse.tile as tile

    f32 = mybir.dt.float32
    f32r = mybir.dt.float32r
    bf16 = mybir.dt.bfloat16
    mult = mybir.AluOpType.mult
    add = mybir.AluOpType.add
    sub = mybir.AluOpType.subtract

    nc = bacc.Bacc("TRN2", target_bir_lowering=False, debug=False,
                   enable_asserts=False, num_devices=NCORES)
    xs_in = nc.dram_tensor("xs", [NB * CL, N, N], f32r, kind="ExternalInput")
    kd_in = nc.dram_tensor("kd", [CL, N, N], f32r, kind="ExternalInput")
    cins = {nm: nc.dram_tensor(nm, list(sh), f32r, kind="ExternalInput")
            for nm, sh in CONST_SHAPES.items()}
    y_out = nc.dram_tensor("y", [N, N], f32, kind="ExternalOutput")

    with tile.TileContext(nc) as tc:
        with tc.tile_pool(name="consts", bufs=1) as cp, \
             tc.tile_pool(name="kf", bufs=1) as kp, \
             tc.tile_pool(name="xio", bufs=2) as xp, \
             tc.tile_pool(name="ud", bufs=2) as udp, \
             tc.tile_pool(name="crt", bufs=8) as crp, \
             tc.tile_pool(name="cast", bufs=4) as csp, \
             tc.tile_pool(name="acc", bufs=2) as ap, \
             tc.tile_pool(name="nyq", bufs=1) as nyp, \
             tc.tile_pool(name="tmp", bufs=3) as tp, \
             tc.tile_pool(name="tmp2", bufs=2) as tp2, \
             tc.tile_pool(name="inv", bufs=1) as ivp, \
             tc.tile_pool(name="psA", bufs=2, space="PSUM") as psA, \
             tc.tile_pool(name="psB", bufs=4, space="PSUM") as psB, \
             tc.tile_pool(name="psN", bufs=2, space="PSUM") as psN, \
             tc.tile_pool(name="dram", bufs=2, space="DRAM") as dp:

            # ---- load constants (chunked along partition) ----
            def load_const(nm, rows, cols):
                ts = []
                nch = (rows + 127) // 128
                for k in range(nch):
                    p = min(128, rows - k * 128)
                    t = cp.tile([p, cols], f32r, name=f"{nm}{k}", tag=f"{nm}{k}")
                    nc.sync.dma_start(t[:], cins[nm][k * 128:k * 128 + p, :])
                    ts.append(t)
                return ts

            Fr = load_const("FrT", N, NE)
            Fi = load_const("FiT", N, NE)
            E2 = [load_const(nm, 256, 128) for nm in ("E2c", "E2p", "E2m")]
            O2 = [load_const(nm, 256, 128) for nm in ("O2c", "O2p", "O2m")]
            alt = load_const("altT", N, 1)
            Ac = load_const("Acos", NQ, N)   # chunks: 128,128,1
            As = load_const("Asin", NQ, N)
            An = load_const("Ansin", NQ, N)
            Bc = load_const("Bcos", 640, N)
            Bs = load_const("Bsin", 640, N)
            o4 = load_const("ones4", CL, 1)

            def prep_image(src):
                """Load one image and compute DIF pre-adds:
                u = xL + xR, d = xL - xR (k-chunk packed)."""
                xt4 = xp.tile([128, 4 * N], f32r, name="xt4", tag="xt4")
                nc.sync.dma_start(
                    xt4[:].rearrange("p (k f) -> p k f", k=4),
                    src.rearrange("(k p) f -> p k f", k=4))
                ud = udp.tile([128, 2048], f32r, name="ud", tag="ud")
                for k in range(4):
                    xL = xt4[:, k * N:k * N + 256]
                    xR = xt4[:, k * N + 256:(k + 1) * N]
                    nc.vector.tensor_tensor(
                        ud[:, k * 256:(k + 1) * 256], xL, xR, op=add)
                    nc.vector.tensor_tensor(
                        ud[:, 1024 + k * 256:1024 + (k + 1) * 256], xL, xR, op=sub)
                return ud

            def forward_image(ud, mode, c, kf, nyqrow, acc=None):
                """ud: prepped image. mode 'k' fills kf tiles for channel c;
                mode 'x' pointwise-accumulates vs kf into acc tiles.
                nyq strip lands in nyqrow[0, c*2NE : c*2NE+2NE] (r|i)."""
                kfr, kfi = kf
                # stage 1 on u and d halves: C^T[n', m], n' in 2 blocks of 128
                cuv = []   # [half][nb] -> (cr, ci)
                for h in range(2):
                    blocks = []
                    for nb in range(2):
                        pr = psA.tile([128, NE], f32, name="ps1r", tag="ps1")
                        pi = psA.tile([128, NE], f32, name="ps1i", tag="ps1")
                        for k in range(4):
                            lhs = ud[:, h * 1024 + k * 256 + nb * 128:
                                     h * 1024 + k * 256 + (nb + 1) * 128]
                            nc.tensor.matmul(pr[:], lhsT=lhs, rhs=Fr[k][:],
                                             start=(k == 0), stop=(k == 3))
                            nc.tensor.matmul(pi[:], lhsT=lhs, rhs=Fi[k][:],
                                             start=(k == 0), stop=(k == 3))
                        cr = crp.tile([128, NE], f32r, name="cr", tag="cr")
                        ci = crp.tile([128, NE], f32r, name="ci", tag="ci")
                        nc.scalar.copy(cr[:], pr[:])
                        nc.scalar.copy(ci[:], pi[:])
                        blocks.append((cr, ci))
                    cuv.append(blocks)

                # stage 2 per parity: contraction over n' (2 chunks)
                for p in range(2):
                    Cc, Cp, Cm = E2 if p == 0 else O2
                    blk = cuv[p]
                    ra = psB.tile([128, NE], f32, name="ra", tag="ps2")
                    ia = psB.tile([128, NE], f32, name="ia", tag="ps2")
                    rb = psB.tile([128, 256], f32, name="rb", tag="ps2")
                    ib = psB.tile([128, 256], f32, name="ib", tag="ps2")
                    for k2 in range(2):
                        st, sp = (k2 == 0), (k2 == 1)
                        cr_, ci_ = blk[k2]
                        a_s = slice(0, NE)
                        b_s = slice(0, 256)
                        nc.tensor.matmul(ra[:], lhsT=Cc[k2][:], rhs=cr_[:, a_s], start=st, stop=False)
                        nc.tensor.matmul(ra[:], lhsT=Cp[k2][:], rhs=ci_[:, a_s], start=False, stop=sp)
                        nc.tensor.matmul(ia[:], lhsT=Cm[k2][:], rhs=cr_[:, a_s], start=st, stop=False)
                        nc.tensor.matmul(ia[:], lhsT=Cc[k2][:], rhs=ci_[:, a_s], start=False, stop=sp)
                        nc.tensor.matmul(rb[:], lhsT=Cc[k2][:], rhs=cr_[:, b_s], start=st, stop=False)
                        nc.tensor.matmul(rb[:], lhsT=Cm[k2][:], rhs=ci_[:, b_s], start=False, stop=sp)
                        nc.tensor.matmul(ib[:], lhsT=Cc[k2][:], rhs=ci_[:, b_s], start=st, stop=False)
                        nc.tensor.matmul(ib[:], lhsT=Cp[k2][:], rhs=cr_[:, b_s], start=False, stop=sp)
                    if mode == "k":
                        # kf plane: [m 0..257 | conj-m 0..255], bf16
                        nc.scalar.copy(kfr[p][:, 0:NE], ra[:])
                        nc.scalar.copy(kfi[p][:, 0:NE], ia[:])
                        nc.scalar.copy(kfr[p][:, NE:W], rb[:])
                        nc.scalar.copy(kfi[p][:, NE:W], ib[:])
                    else:
                        # cast spectra to bf16 for the 2x-mode pointwise
                        xa_r = csp.tile([128, NE], bf16, name="xar", tag="xa")
                        xa_i = csp.tile([128, NE], bf16, name="xai", tag="xa")
                        xb_r = csp.tile([128, 256], bf16, name="xbr", tag="xb")
                        xb_i = csp.tile([128, 256], bf16, name="xbi", tag="xb")
                        nc.scalar.copy(xa_r[:], ra[:])
                        nc.scalar.copy(xa_i[:], ia[:])
                        nc.scalar.copy(xb_r[:], rb[:])
                        nc.scalar.copy(xb_i[:], ib[:])
                        # acc layout: [128, 4W] = [p0:(ar|ai) p1:(ar|ai)]
                        arp = acc[:, p * 2 * W:p * 2 * W + W]
                        aip = acc[:, p * 2 * W + W:(p + 1) * 2 * W]
                        for (xr_, xi_, cols) in ((xa_r, xa_i, slice(0, NE)),
                                                 (xb_r, xb_i, slice(NE, W))):
                            kr = kfr[p][:, cols]
                            ki = kfi[p][:, cols]
                            w_ = cols.stop - cols.start
                            t1 = tp.tile([128, NE], bf16, name="t1", tag="t1")
                            t2 = tp.tile([128, NE], bf16, name="t2", tag="t2")
                            nc.vector.tensor_tensor(t1[:, :w_], xr_[:], kr, op=mult)
                            nc.vector.tensor_tensor(t2[:, :w_], xi_[:], ki, op=mult)
                            if c == 0:
                                nc.vector.tensor_tensor(arp[:, cols], t1[:, :w_], t2[:, :w_], op=sub)
                            else:
                                u = tp.tile([128, NE], bf16, name="u", tag="u")
                                nc.vector.tensor_tensor(u[:, :w_], t1[:, :w_], t2[:, :w_], op=sub)
                                nc.vector.tensor_tensor(arp[:, cols], arp[:, cols], u[:, :w_], op=add)
                            nc.vector.tensor_tensor(t1[:, :w_], xr_[:], ki, op=mult)
                            nc.vector.tensor_tensor(t2[:, :w_], xi_[:], kr, op=mult)
                            if c == 0:
                                nc.vector.tensor_tensor(aip[:, cols], t1[:, :w_], t2[:, :w_], op=add)
                            else:
                                u = tp.tile([128, NE], bf16, name="u", tag="u")
                                nc.vector.tensor_tensor(u[:, :w_], t1[:, :w_], t2[:, :w_], op=add)
                                nc.vector.tensor_tensor(aip[:, cols], aip[:, cols], u[:, :w_], op=add)

                # nyquist strip q=256 (contract u-planes with (-1)^n')
                nr = psN.tile([1, NE], f32, name="nr", tag="psn")
                ni = psN.tile([1, NE], f32, name="ni", tag="psn")
                for k2 in range(2):
                    st, sp = (k2 == 0), (k2 == 1)
                    cr_, ci_ = cuv[0][k2]
                    nc.tensor.matmul(nr[:], lhsT=alt[0][:], rhs=cr_[:], start=st, stop=sp)
                    nc.tensor.matmul(ni[:], lhsT=alt[0][:], rhs=ci_[:], start=st, stop=sp)
                nc.scalar.copy(nyqrow[0:1, c * 2 * NE:c * 2 * NE + NE], nr[:])
                nc.scalar.copy(nyqrow[0:1, c * 2 * NE + NE:(c + 1) * 2 * NE], ni[:])

            def inverse(rs_out):
                T4 = ivp.tile([128, 4 * W], f32r, name="T4", tag="T4")
                if RS_F32:
                    nc.gpsimd.dma_start(
                        T4[:], rs_out[0:4 * 128 * W].rearrange("(p f) -> p f", p=128))
                else:
                    T4b = ivp.tile([128, 4 * W], bf16, name="T4b", tag="T4b")
                    nc.gpsimd.dma_start(
                        T4b[:], rs_out[0:4 * 128 * W].rearrange("(p f) -> p f", p=128))
                    nc.scalar.copy(T4[:], T4b[:])
                Tr = [T4[:, 0:W], T4[:, 2 * W:3 * W]]
                Ti = [T4[:, W:2 * W], T4[:, 3 * W:4 * W]]
                tb = ivp.tile([1, 2 * NE], f32 if RS_F32 else bf16,
                              name="tb", tag="tb")
                tnr = ivp.tile([1, N], f32r, name="tnr", tag="tnr")
                tni = ivp.tile([1, N], f32r, name="tni", tag="tni")
                nyb = 4 * 128 * W
                nc.gpsimd.dma_start(tb[:], rs_out[nyb:nyb + 2 * NE].rearrange("(p f) -> p f", p=1))
                nc.scalar.copy(tnr[:, 0:NE], tb[:, 0:NE])
                nc.scalar.copy(tni[:, 0:NE], tb[:, NE:2 * NE])
                # nyq fixup: T[256, 257:512] = T[256, 1:256]
                nc.vector.tensor_copy(tnr[:, NQ:N], tnr[:, 1:256])
                nc.vector.tensor_copy(tni[:, NQ:N], tni[:, 1:256])

                # inv stage 1: R[j, n] per j-chunk. The packed plane stores
                # j<=256 at col j, j>=257 at col j+2 (cs), strip at j direct.
                jsl = [(slice(0, 128), slice(0, 128), 128, True),
                       (slice(128, 256), slice(128, 256), 128, True),
                       (slice(257, 384), slice(259, 386), 127, False),
                       (slice(384, 512), slice(386, 514), 128, False)]
                Rr, Ri = [], []
                for (js, cs, mw, plus) in jsl:
                    prr = psB.tile([mw, N], f32, name="prr", tag="ps2")
                    pri = psB.tile([mw, N], f32, name="pri", tag="ps2")
                    for qk in range(3):       # q chunks: 128,128,1(nyq strip)
                        st, sp = (qk == 0), (qk == 2)
                        if qk < 2:
                            lr, li = Tr[qk][:, cs], Ti[qk][:, cs]
                        else:
                            lr, li = tnr[:, js], tni[:, js]
                        # Rr = Tr.Acos -/+ Ti.Asin ; sign folded via const choice
                        nc.tensor.matmul(prr[:], lhsT=lr, rhs=Ac[qk][:], start=st, stop=False)
                        nc.tensor.matmul(prr[:], lhsT=li, rhs=(An if plus else As)[qk][:], start=False, stop=sp)
                        # Ri = Tr.(+/-Asin) + Ti.Acos
                        nc.tensor.matmul(pri[:], lhsT=lr, rhs=(As if plus else An)[qk][:], start=st, stop=False)
                        nc.tensor.matmul(pri[:], lhsT=li, rhs=Ac[qk][:], start=False, stop=sp)
                    rr = ivp.tile([mw, N], f32r, name="rr", tag=f"rr{js.start}")
                    ri = ivp.tile([mw, N], f32r, name="ri", tag=f"ri{js.start}")
                    nc.scalar.copy(rr[:], prr[:])
                    nc.scalar.copy(ri[:], pri[:])
                    Rr.append(rr)
                    Ri.append(ri)
                # j=256 column strip (uses '+' signs)
                p6r = psN.tile([1, N], f32, name="p6r", tag="psn")
                p6i = psN.tile([1, N], f32, name="p6i", tag="psn")
                for qk in range(3):
                    st, sp = (qk == 0), (qk == 2)
                    if qk < 2:
                        lr, li = Tr[qk][:, 256:257], Ti[qk][:, 256:257]
                    else:
                        lr, li = tnr[:, 256:257], tni[:, 256:257]
                    nc.tensor.matmul(p6r[:], lhsT=lr, rhs=Ac[qk][:], start=st, stop=False)
                    nc.tensor.matmul(p6r[:], lhsT=li, rhs=An[qk][:], start=False, stop=sp)
                    nc.tensor.matmul(p6i[:], lhsT=lr, rhs=As[qk][:], start=st, stop=False)
                    nc.tensor.matmul(p6i[:], lhsT=li, rhs=Ac[qk][:], start=False, stop=sp)
                r6r = ivp.tile([1, N], f32r, name="r6r", tag="r6r")
                r6i = ivp.tile([1, N], f32r, name="r6i", tag="r6i")
                nc.scalar.copy(r6r[:], p6r[:])
                nc.scalar.copy(r6i[:], p6i[:])

                # inv stage 2: y[r, n] = sum_j Bcos[j,r].Rr[j,n] + Bsin[j,r].Ri[j,n]
                for rc in range(4):
                    rs = slice(rc * 128, (rc + 1) * 128)
                    py = psB.tile([128, N], f32, name="py", tag="ps2")
                    kws = [128, 128, 127, 128]
                    for ji in range(5):
                        st = (ji == 0)
                        if ji < 4:
                            kw = kws[ji]
                            nc.tensor.matmul(py[:], lhsT=Bc[ji][0:kw, rs], rhs=Rr[ji][:], start=st, stop=False)
                            nc.tensor.matmul(py[:], lhsT=Bs[ji][0:kw, rs], rhs=Ri[ji][:], start=False, stop=False)
                        else:
                            nc.tensor.matmul(py[:], lhsT=Bc[4][0:1, rs], rhs=r6r[:], start=False, stop=False)
                            nc.tensor.matmul(py[:], lhsT=Bs[4][0:1, rs], rhs=r6i[:], start=False, stop=True)
                    yt = tp2.tile([128, N], f32, name="yt", tag="yt")
                    nc.scalar.copy(yt[:], py[:])
                    nc.scalar.dma_start(y_out[rs, :], yt[:])

            # image stream with one-ahead prefetch (load + pre-adds) so the
            # next image's DVE prep lands before the current image's cmul
            stream = []
            for _rep in range(repeat):
                stream += [kd_in[c] for c in range(CL)]
                stream += [xs_in[b * CL + c]
                           for b in range(NB) for c in range(CL)]
            pstate = {"i": 0, "ud": prep_image(stream[0])}

            def next_image():
                i = pstate["i"]
                ud = pstate["ud"]
                if i + 1 < len(stream):
                    pstate["ud"] = prep_image(stream[i + 1])
                pstate["i"] = i + 1
                return ud

            prev_rs_out = None
            for _rep in range(repeat):
                rdt = f32 if RS_F32 else bf16
                rs_in = dp.tile([NCORES, PB], rdt, name="rs_in", tag="rs_in")
                rs_out = dp.tile([PB], rdt, name="rs_out", tag="rs_out")
                # ---- phase 1: kernel channels ----
                kfr = [[kp.tile([128, W], bf16, name=f"kfr{c}_{qc}", tag=f"kfr{c}_{qc}")
                        for qc in range(2)] for c in range(CL)]
                kfi = [[kp.tile([128, W], bf16, name=f"kfi{c}_{qc}", tag=f"kfi{c}_{qc}")
                        for qc in range(2)] for c in range(CL)]
                nyqk = nyp.tile([1, CL * 2 * NE], f32, name="nyqk", tag="nyqrow")
                for c in range(CL):
                    forward_image(next_image(), "k", c, (kfr[c], kfi[c]), nyqk)
                kfn = kp.tile([CL, 2 * NE], f32, name="kfn", tag="kfn")
                nydk = dp.tile([CL * 2 * NE], f32, name="nydk", tag="nyd")
                nc.gpsimd.dma_start(
                    nydk[:].rearrange("(p f) -> p f", p=1), nyqk[0:1, :])
                nc.gpsimd.dma_start(
                    kfn[:], nydk[:].rearrange("(c f) -> c f", c=CL))

                # ---- phase 2: batches (prev iteration's inverse after b1) ----
                for b in range(NB):
                    acc = ap.tile([128, 4 * W], bf16, name="acc", tag="acc")
                    nyqx = nyp.tile([1, CL * 2 * NE], f32, name="nyqx", tag="nyqrow")
                    for c in range(CL):
                        forward_image(next_image(), "x", c,
                                      (kfr[c], kfi[c]), nyqx, acc=acc)
                    xn = ap.tile([CL, 2 * NE], f32, name="xn", tag="xn")
                    nydx = dp.tile([CL * 2 * NE], f32, name="nydx", tag="nyd")
                    nc.gpsimd.dma_start(
                        nydx[:].rearrange("(p f) -> p f", p=1), nyqx[0:1, :])
                    nc.gpsimd.dma_start(
                        xn[:], nydx[:].rearrange("(c f) -> c f", c=CL))
                    # nyq pointwise (per-channel rows) + channel reduce via K=4 matmul
                    xnr, xni = xn[:, 0:NE], xn[:, NE:2 * NE]
                    kfnr, kfni = kfn[:, 0:NE], kfn[:, NE:2 * NE]
                    t1 = tp2.tile([CL, NE], f32, name="nt1", tag="nt")
                    t2 = tp2.tile([CL, NE], f32, name="nt2", tag="nt")
                    un = tp2.tile([CL, NE], f32r, name="un", tag="un")
                    vn = tp2.tile([CL, NE], f32r, name="vn", tag="un")
                    nc.vector.tensor_tensor(t1[:], xnr, kfnr, op=mult)
                    nc.vector.tensor_tensor(t2[:], xni, kfni, op=mult)
                    nc.vector.tensor_tensor(un[:], t1[:], t2[:], op=sub)
                    nc.vector.tensor_tensor(t1[:], xnr, kfni, op=mult)
                    nc.vector.tensor_tensor(t2[:], xni, kfnr, op=mult)
                    nc.vector.tensor_tensor(vn[:], t1[:], t2[:], op=add)
                    pnr = psN.tile([1, NE], f32, name="pnr", tag="psn")
                    pni = psN.tile([1, NE], f32, name="pni", tag="psn")
                    nc.tensor.matmul(pnr[:], lhsT=o4[0][:], rhs=un[:], start=True, stop=True)
                    nc.tensor.matmul(pni[:], lhsT=o4[0][:], rhs=vn[:], start=True, stop=True)
                    sn2 = tp2.tile([1, 2 * NE], f32 if RS_F32 else bf16,
                                   name="sn2", tag="sn")
                    nc.scalar.copy(sn2[:, 0:NE], pnr[:])
                    nc.scalar.copy(sn2[:, NE:2 * NE], pni[:])
                    # ship batch partial to rs_in row b (single big DMA + nyq)
                    row = rs_in[b]
                    if RS_F32:
                        accf = nyp.tile([128, 4 * W], f32, name="accf", tag="accf")
                        nc.scalar.copy(accf[:], acc[:])
                        src_acc = accf
                    else:
                        src_acc = acc
                    nc.gpsimd.dma_start(
                        row[0:4 * 128 * W].rearrange("(p f) -> p f", p=128),
                        src_acc[:])
                    nyb = 4 * 128 * W
                    nc.gpsimd.dma_start(
                        row[nyb:nyb + 2 * NE].rearrange("(p f) -> p f", p=1),
                        sn2[:])
                    if b == 1 and prev_rs_out is not None:
                        inverse(prev_rs_out)

                # ---- phase 3: reduce-scatter ----
                nc.gpsimd.collective_compute(
                    "ReduceScatter", mybir.AluOpType.add,
                    replica_groups=[list(range(NCORES))],
                    ins=[rs_in[:].opt()], outs=[rs_out[:].opt()],
                )
                prev_rs_out = rs_out

            # drain: last iteration's inverse
            inverse(prev_rs_out)

    nc.compile()
    return nc


def _get_nc(repeat=1):
    key = f"nc{repeat}"
    if key not in _CACHE:
        _CACHE[key] = _build_nc(repeat)
    return _CACHE[key]


def _get_runner(repeat=1):
    """Build (once) and cache a jitted SPMD executable for the bass program.

    Replicates concourse.bass2jax.run_bass_via_pjrt, but keeps the jitted
    callable alive across kernel() invocations so repeated calls reuse the
    compiled NEFF instead of re-tracing + re-compiling every time.
    """
    key = f"runner{repeat}"
    if key in _CACHE:
        return _CACHE[key]
    import jax
    from jax.sharding import Mesh, PartitionSpec
    from jax.experimental.shard_map import shard_map
    import concourse.bass2jax as b2j
    import concourse.mybir as mybir

    b2j.install_neuronx_cc_hook()
    nc = _get_nc(repeat)
    assert nc.dbg_addr is None
    partition_name = (nc.partition_id_tensor.name
                      if nc.partition_id_tensor else None)
    in_names, out_names, out_avals, zero_shapes = [], [], [], []
    for alloc in nc.m.functions[0].allocations:
        if not isinstance(alloc, mybir.MemoryLocationSet):
            continue
        name = alloc.memorylocations[0].name
        if alloc.kind == "ExternalInput":
            if name != partition_name:
                in_names.append(name)
        elif alloc.kind == "ExternalOutput":
            shape = tuple(alloc.tensor_shape)
            dtype = mybir.dt.np(alloc.dtype)
            out_names.append(name)
            out_avals.append(jax.core.ShapedArray(shape, dtype))
            zero_shapes.append((shape, dtype))
    n_params = len(in_names)
    n_outs = len(out_avals)
    all_in_names = list(in_names) + list(out_names)
    if partition_name is not None:
        all_in_names.append(partition_name)
    donate = tuple(range(n_params, n_params + n_outs))

    def _body(*args):
        operands = list(args)
        if partition_name is not None:
            operands.append(b2j.partition_id_tensor())
        outs = b2j._bass_exec_p.bind(
            *operands,
            out_avals=tuple(out_avals),
            in_names=tuple(all_in_names),
            out_names=tuple(out_names),
            lowering_input_output_aliases=(),
            sim_require_finite=True,
            sim_require_nnan=True,
            nc=nc,
        )
        return tuple(outs)

    devices = jax.devices()[:NCORES]
    mesh = Mesh(np.asarray(devices), ("core",))
    in_specs = (PartitionSpec("core"),) * (n_params + n_outs)
    out_specs = (PartitionSpec("core"),) * n_outs
    sharded = jax.jit(
        shard_map(_body, mesh=mesh, in_specs=in_specs,
                  out_specs=out_specs, check_rep=False),
        donate_argnums=donate, keep_unused=True,
    )
    runner = (sharded, in_names, out_names, out_avals, zero_shapes)
    _CACHE[key] = runner
    return runner


def _run_spmd(in_maps, repeat=1):
    sharded, in_names, out_names, out_avals, zero_shapes = _get_runner(repeat)
    concat_in = [
        np.concatenate([np.asarray(in_maps[c][name]) for c in range(NCORES)],
                       axis=0)
        for name in in_names
    ]
    concat_zeros = [
        np.zeros((NCORES * sh[0], *sh[1:]), dt) for (sh, dt) in zero_shapes
    ]
    out_arrs = sharded(*concat_in, *concat_zeros)
    return [
        {name: np.asarray(out_arrs[i]).reshape(NCORES, *out_avals[i].shape)[c]
         for i, name in enumerate(out_names)}
        for c in range(NCORES)
    ]


def build_in_maps(x, kernel, variable_kernel, loc_idx):
    x = np.asarray(x)
    vk = np.asarray(variable_kernel)
    idx = np.asarray(loc_idx)
    # host scatter of relu'd values into the dense PSF (data movement only)
    kflat = np.asarray(kernel)[0].reshape(-1).copy()
    kflat[idx] = np.maximum(vk, 0.0).astype(np.float32)
    kd_all = kflat.reshape(NC_TOT, N, N)

    C = _consts()
    in_maps = []
    for core in range(NCORES):
        c0 = core * CL
        m = {nm: C[nm] for nm in CONST_SHAPES}
        m["xs"] = np.ascontiguousarray(
            x[:, c0:c0 + CL].reshape(NB * CL, N, N)).astype(np.float32)
        m["kd"] = np.ascontiguousarray(kd_all[c0:c0 + CL]).astype(np.float32)
        in_maps.append(m)
    return in_maps


def kernel(x, kernel, variable_kernel, loc_idx, _repeat=1):
    in_maps = build_in_maps(x, kernel, variable_kernel, loc_idx)
    res = _run_spmd(in_maps, _repeat)
    out = np.stack([res[b]["y"] for b in range(NB)], axis=0)
    return out.astype(np.float32)


# revision 37
# speedup vs baseline: 5062.6172x; 1.3029x over previous
"""Trainium2 Bass kernel: per-channel circular conv via DFT matmuls, summed
over channels (sparse PSF kernel), 8-core channel-sharded SPMD.

out[b] = irfft2( sum_c rfft2(x[b,c]) * rfft2(scatter(relu(vk), idx)[c]) )

Sharding: each core owns 4 of 32 channels (forward FFTs + pointwise
multiply-accumulate), ReduceScatter(add) over batch gives core b the summed
spectrum of batch b, which it inverse-transforms. All FFTs are dense DFT
matmuls in float32r (full PE rate at moving-dim >= 256, even N required).

Spectra are kept transposed ("T-form", [q (0..256) x j (0..511)]) with the
m>256 half stored conjugated at its natural compute position ("P-form") so
no data reversal is ever needed - all permutations/conjugations/signs are
absorbed into host-precomputed constant matrices, including the inverse.
"""
import numpy as np

N = 512
NQ = 257
NE = 258          # even-padded 257 (fp32r matmul needs even moving dim)
W = 514           # packed plane width: [m 0..257 | conj-m 0..255]
NB = 8            # batches (one per core after reduce-scatter)
CL = 4            # channels per core
NC_TOT = 32
NCORES = 8
TH = 2 * np.pi / N
PB = 4 * 128 * W + 2 * NE  # per-batch rs payload (bf16): 4 planes + nyq r|i

RS_F32 = False
_CACHE = {}


def _consts():
    if "consts" in _CACHE:
        return _CACHE["consts"]
    r = np.arange(N)
    m = np.arange(NQ)
    ang1 = TH * np.outer(r, m)
    FrT = np.zeros((N, NE), np.float32)
    FiT = np.zeros((N, NE), np.float32)
    FrT[:, :NQ] = np.cos(ang1)
    FiT[:, :NQ] = -np.sin(ang1)
    n2 = np.arange(256)
    q2 = np.arange(128)
    ang_e = 2 * np.pi * np.outer(n2, q2) / 256.0
    ang_o = 2 * np.pi * np.outer(n2, 2 * q2 + 1) / 512.0
    E2c = np.cos(ang_e).astype(np.float32)
    E2p = np.sin(ang_e).astype(np.float32)
    E2m = -E2p
    O2c = np.cos(ang_o).astype(np.float32)
    O2p = np.sin(ang_o).astype(np.float32)
    O2m = -O2p
    altT = ((-1.0) ** r).astype(np.float32).reshape(N, 1)
    w = np.full(NQ, 2.0)
    w[0] = 1.0
    w[256] = 1.0
    angA = TH * np.outer(np.arange(NQ), r)
    # rows permuted to DIF order: even q, odd q, then q=256 strip
    qperm = np.concatenate([np.arange(0, 256, 2), np.arange(1, 256, 2), [256]])
    Acos = (w[:, None] * np.cos(angA)).astype(np.float32)[qperm]
    Asin = (w[:, None] * np.sin(angA)).astype(np.float32)[qperm]
    Ansin = -Asin
    j = np.arange(N)
    angB = TH * np.outer(j, r)
    sgn = np.ones((N, N))
    sgn[257:, :] = ((-1.0) ** r)[None, :]
    Bcos_t = (np.cos(angB) * sgn / (N * N)).astype(np.float32)
    Bsin_t = (-np.sin(angB) * sgn / (N * N)).astype(np.float32)

    def bpack(Bm):
        out = np.zeros((640, N), np.float32)
        out[0:128] = Bm[0:128]
        out[128:256] = Bm[128:256]
        out[256:256 + 127] = Bm[257:384]
        out[384:512] = Bm[384:512]
        out[512:513] = Bm[256:257]
        return out
    Bcos = bpack(Bcos_t)
    Bsin = bpack(Bsin_t)
    ones4 = np.ones((CL, 1), np.float32)
    out = dict(FrT=FrT, FiT=FiT, E2c=E2c, E2p=E2p, E2m=E2m,
               O2c=O2c, O2p=O2p, O2m=O2m, altT=altT,
               Acos=Acos, Asin=Asin, Ansin=Ansin, Bcos=Bcos, Bsin=Bsin,
               ones4=ones4)
    _CACHE["consts"] = out
    return out


CONST_SHAPES = dict(FrT=(N, NE), FiT=(N, NE), E2c=(256, 128), E2p=(256, 128),
                    E2m=(256, 128), O2c=(256, 128), O2p=(256, 128),
                    O2m=(256, 128), altT=(N, 1), Acos=(NQ, N), Asin=(NQ, N),
                    Ansin=(NQ, N), Bcos=(640, N), Bsin=(640, N), ones4=(CL, 1))


def _build_nc(repeat=1):
    import concourse.bacc as bacc
    import concourse.mybir as mybir
    import concourse.tile as tile

    f32 = mybir.dt.float32
    f32r = mybir.dt.float32r
    bf16 = mybir.dt.bfloat16
    mult = mybir.AluOpType.mult
    add = mybir.AluOpType.add
    sub = mybir.AluOpType.subtract

    nc = bacc.Bacc("TRN2", target_bir_lowering=False, debug=False,
                   enable_asserts=False, num_devices=NCORES)
    xs_in = nc.dram_tensor("xs", [NB * CL, N, N], f32r, kind="ExternalInput")
    kd_in = nc.dram_tensor("kd", [CL, N, N], f32r, kind="ExternalInput")
    cins = {nm: nc.dram_tensor(nm, list(sh), f32r, kind="ExternalInput")
            for nm, sh in CONST_SHAPES.items()}
    y_out = nc.dram_tensor("y", [N, N], f32, kind="ExternalOutput")

    with tile.TileContext(nc) as tc:
        with tc.tile_pool(name="consts", bufs=1) as cp, \
             tc.tile_pool(name="kf", bufs=1) as kp, \
             tc.tile_pool(name="xio", bufs=2) as xp, \
             tc.tile_pool(name="ud", bufs=2) as udp, \
             tc.tile_pool(name="crt", bufs=8) as crp, \
             tc.tile_pool(name="cast", bufs=4) as csp, \
             tc.tile_pool(name="acc", bufs=2) as ap, \
             tc.tile_pool(name="nyq", bufs=1) as nyp, \
             tc.tile_pool(name="tmp", bufs=3) as tp, \
             tc.tile_pool(name="tmp2", bufs=2) as tp2, \
             tc.tile_pool(name="inv", bufs=1) as ivp, \
             tc.tile_pool(name="psA", bufs=2, space="PSUM") as psA, \
             tc.tile_pool(name="psB", bufs=4, space="PSUM") as psB, \
             tc.tile_pool(name="psN", bufs=2, space="PSUM") as psN, \
             tc.tile_pool(name="dram", bufs=2, space="DRAM") as dp:

            # ---- load constants (chunked along partition) ----
            def load_const(nm, rows, cols):
                ts = []
                nch = (rows + 127) // 128
                for k in range(nch):
                    p = min(128, rows - k * 128)
                    t = cp.tile([p, cols], f32r, name=f"{nm}{k}", tag=f"{nm}{k}")
                    nc.sync.dma_start(t[:], cins[nm][k * 128:k * 128 + p, :])
                    ts.append(t)
                return ts

            Fr = load_const("FrT", N, NE)
            Fi = load_const("FiT", N, NE)
            E2 = [load_const(nm, 256, 128) for nm in ("E2c", "E2p", "E2m")]
            O2 = [load_const(nm, 256, 128) for nm in ("O2c", "O2p", "O2m")]
            alt = load_const("altT", N, 1)
            Ac = load_const("Acos", NQ, N)   # chunks: 128,128,1
            As = load_const("Asin", NQ, N)
            An = load_const("Ansin", NQ, N)
            Bc = load_const("Bcos", 640, N)
            Bs = load_const("Bsin", 640, N)
            o4 = load_const("ones4", CL, 1)

            def prep_image(src):
                """Load one image and compute DIF pre-adds:
                u = xL + xR, d = xL - xR (k-chunk packed)."""
                xt4 = xp.tile([128, 4 * N], f32r, name="xt4", tag="xt4")
                nc.sync.dma_start(
                    xt4[:].rearrange("p (k f) -> p k f", k=4),
                    src.rearrange("(k p) f -> p k f", k=4))
                ud = udp.tile([128, 2048], f32r, name="ud", tag="ud")
                for k in range(4):
                    xL = xt4[:, k * N:k * N + 256]
                    xR = xt4[:, k * N + 256:(k + 1) * N]
                    nc.vector.tensor_tensor(
                        ud[:, k * 256:(k + 1) * 256], xL, xR, op=add)
                    nc.vector.tensor_tensor(
                        ud[:, 1024 + k * 256:1024 + (k + 1) * 256], xL, xR, op=sub)
                return ud

            def forward_image(ud, mode, c, kf, nyqrow, acc=None):
                """ud: prepped image. mode 'k' fills kf tiles for channel c;
                mode 'x' pointwise-accumulates vs kf into acc tiles.
                nyq strip lands in nyqrow[0, c*2NE : c*2NE+2NE] (r|i)."""
                kfr, kfi = kf
                # stage 1 on u and d halves: C^T[n', m], n' in 2 blocks of 128
                cuv = []   # [half][nb] -> (cr, ci)
                for h in range(2):
                    blocks = []
                    for nb in range(2):
                        pr = psA.tile([128, NE], f32, name="ps1r", tag="ps1")
                        pi = psA.tile([128, NE], f32, name="ps1i", tag="ps1")
                        for k in range(4):
                            lhs = ud[:, h * 1024 + k * 256 + nb * 128:
                                     h * 1024 + k * 256 + (nb + 1) * 128]
                            nc.tensor.matmul(pr[:], lhsT=lhs, rhs=Fr[k][:],
                                             start=(k == 0), stop=(k == 3))
                            nc.tensor.matmul(pi[:], lhsT=lhs, rhs=Fi[k][:],
                                             start=(k == 0), stop=(k == 3))
                        cr = crp.tile([128, NE], f32r, name="cr", tag="cr")
                        ci = crp.tile([128, NE], f32r, name="ci", tag="ci")
                        nc.scalar.copy(cr[:], pr[:])
                        nc.scalar.copy(ci[:], pi[:])
                        blocks.append((cr, ci))
                    cuv.append(blocks)

                # stage 2 per parity: contraction over n' (2 chunks)
                for p in range(2):
                    Cc, Cp, Cm = E2 if p == 0 else O2
                    blk = cuv[p]
                    ra = psB.tile([128, NE], f32, name="ra", tag="ps2")
                    ia = psB.tile([128, NE], f32, name="ia", tag="ps2")
                    rb = psB.tile([128, 256], f32, name="rb", tag="ps2")
                    ib = psB.tile([128, 256], f32, name="ib", tag="ps2")
                    for k2 in range(2):
                        st, sp = (k2 == 0), (k2 == 1)
                        cr_, ci_ = blk[k2]
                        a_s = slice(0, NE)
                        b_s = slice(0, 256)
                        nc.tensor.matmul(ra[:], lhsT=Cc[k2][:], rhs=cr_[:, a_s], start=st, stop=False)
                        nc.tensor.matmul(ra[:], lhsT=Cp[k2][:], rhs=ci_[:, a_s], start=False, stop=sp)
                        nc.tensor.matmul(ia[:], lhsT=Cm[k2][:], rhs=cr_[:, a_s], start=st, stop=False)
                        nc.tensor.matmul(ia[:], lhsT=Cc[k2][:], rhs=ci_[:, a_s], start=False, stop=sp)
                        nc.tensor.matmul(rb[:], lhsT=Cc[k2][:], rhs=cr_[:, b_s], start=st, stop=False)
                        nc.tensor.matmul(rb[:], lhsT=Cm[k2][:], rhs=ci_[:, b_s], start=False, stop=sp)
                        nc.tensor.matmul(ib[:], lhsT=Cc[k2][:], rhs=ci_[:, b_s], start=st, stop=False)
                        nc.tensor.matmul(ib[:], lhsT=Cp[k2][:], rhs=cr_[:, b_s], start=False, stop=sp)
                    if mode == "k":
                        # kf plane: [m 0..257 | conj-m 0..255], bf16
                        nc.scalar.copy(kfr[p][:, 0:NE], ra[:])
                        nc.scalar.copy(kfi[p][:, 0:NE], ia[:])
                        nc.scalar.copy(kfr[p][:, NE:W], rb[:])
                        nc.scalar.copy(kfi[p][:, NE:W], ib[:])
                    else:
                        # cast spectra to bf16 for the 2x-mode pointwise
                        xa_r = csp.tile([128, NE], bf16, name="xar", tag="xa")
                        xa_i = csp.tile([128, NE], bf16, name="xai", tag="xa")
                        xb_r = csp.tile([128, 256], bf16, name="xbr", tag="xb")
                        xb_i = csp.tile([128, 256], bf16, name="xbi", tag="xb")
                        nc.scalar.copy(xa_r[:], ra[:])
                        nc.scalar.copy(xa_i[:], ia[:])
                        nc.scalar.copy(xb_r[:], rb[:])
                        nc.scalar.copy(xb_i[:], ib[:])
                        # acc layout: [128, 4W] = [p0:(ar|ai) p1:(ar|ai)]
                        arp = acc[:, p * 2 * W:p * 2 * W + W]
                        aip = acc[:, p * 2 * W + W:(p + 1) * 2 * W]
                        for (xr_, xi_, cols) in ((xa_r, xa_i, slice(0, NE)),
                                                 (xb_r, xb_i, slice(NE, W))):
                            kr = kfr[p][:, cols]
                            ki = kfi[p][:, cols]
                            w_ = cols.stop - cols.start
                            t1 = tp.tile([128, NE], bf16, name="t1", tag="t1")
                            t2 = tp.tile([128, NE], bf16, name="t2", tag="t2")
                            nc.vector.tensor_tensor(t1[:, :w_], xr_[:], kr, op=mult)
                            nc.vector.tensor_tensor(t2[:, :w_], xi_[:], ki, op=mult)
                            if c == 0:
                                nc.vector.tensor_tensor(arp[:, cols], t1[:, :w_], t2[:, :w_], op=sub)
                            else:
                                u = tp.tile([128, NE], bf16, name="u", tag="u")
                                nc.vector.tensor_tensor(u[:, :w_], t1[:, :w_], t2[:, :w_], op=sub)
                                nc.vector.tensor_tensor(arp[:, cols], arp[:, cols], u[:, :w_], op=add)
                            nc.vector.tensor_tensor(t1[:, :w_], xr_[:], ki, op=mult)
                            nc.vector.tensor_tensor(t2[:, :w_], xi_[:], kr, op=mult)
                            if c == 0:
                                nc.vector.tensor_tensor(aip[:, cols], t1[:, :w_], t2[:, :w_], op=add)
                            else:
                                u = tp.tile([128, NE], bf16, name="u", tag="u")
                                nc.vector.tensor_tensor(u[:, :w_], t1[:, :w_], t2[:, :w_], op=add)
                                nc.vector.tensor_tensor(aip[:, cols], aip[:, cols], u[:, :w_], op=add)

                # nyquist strip q=256 (contract u-planes with (-1)^n')
                nr = psN.tile([1, NE], f32, name="nr", tag="psn")
                ni = psN.tile([1, NE], f32, name="ni", tag="psn")
                for k2 in range(2):
                    st, sp = (k2 == 0), (k2 == 1)
                    cr_, ci_ = cuv[0][k2]
                    nc.tensor.matmul(nr[:], lhsT=alt[0][:], rhs=cr_[:], start=st, stop=sp)
                    nc.tensor.matmul(ni[:], lhsT=alt[0][:], rhs=ci_[:], start=st, stop=sp)
                nc.scalar.copy(nyqrow[0:1, c * 2 * NE:c * 2 * NE + NE], nr[:])
                nc.scalar.copy(nyqrow[0:1, c * 2 * NE + NE:(c + 1) * 2 * NE], ni[:])

            def inverse(rs_out):
                T4 = ivp.tile([128, 4 * W], f32r, name="T4", tag="T4")
                if RS_F32:
                    nc.gpsimd.dma_start(
                        T4[:], rs_out[0:4 * 128 * W].rearrange("(p f) -> p f", p=128))
                else:
                    T4b = ivp.tile([128, 4 * W], bf16, name="T4b", tag="T4b")
                    nc.gpsimd.dma_start(
                        T4b[:], rs_out[0:4 * 128 * W].rearrange("(p f) -> p f", p=128))
                    nc.scalar.copy(T4[:], T4b[:])
                Tr = [T4[:, 0:W], T4[:, 2 * W:3 * W]]
                Ti = [T4[:, W:2 * W], T4[:, 3 * W:4 * W]]
                tb = ivp.tile([1, 2 * NE], f32 if RS_F32 else bf16,
                              name="tb", tag="tb")
                tnr = ivp.tile([1, N], f32r, name="tnr", tag="tnr")
                tni = ivp.tile([1, N], f32r, name="tni", tag="tni")
                nyb = 4 * 128 * W
                nc.gpsimd.dma_start(tb[:], rs_out[nyb:nyb + 2 * NE].rearrange("(p f) -> p f", p=1))
                nc.scalar.copy(tnr[:, 0:NE], tb[:, 0:NE])
                nc.scalar.copy(tni[:, 0:NE], tb[:, NE:2 * NE])
                # nyq fixup: T[256, 257:512] = T[256, 1:256]
                nc.vector.tensor_copy(tnr[:, NQ:N], tnr[:, 1:256])
                nc.vector.tensor_copy(tni[:, NQ:N], tni[:, 1:256])

                # inv stage 1: R[j, n] per j-chunk. The packed plane stores
                # j<=256 at col j, j>=257 at col j+2 (cs), strip at j direct.
                jsl = [(slice(0, 128), slice(0, 128), 128, True),
                       (slice(128, 256), slice(128, 256), 128, True),
                       (slice(257, 384), slice(259, 386), 127, False),
                       (slice(384, 512), slice(386, 514), 128, False)]
                Rr, Ri = [], []
                for (js, cs, mw, plus) in jsl:
                    prr = psB.tile([mw, N], f32, name="prr", tag="ps2")
                    pri = psB.tile([mw, N], f32, name="pri", tag="ps2")
                    for qk in range(3):       # q chunks: 128,128,1(nyq strip)
                        st, sp = (qk == 0), (qk == 2)
                        if qk < 2:
                            lr, li = Tr[qk][:, cs], Ti[qk][:, cs]
                        else:
                            lr, li = tnr[:, js], tni[:, js]
                        # Rr = Tr.Acos -/+ Ti.Asin ; sign folded via const choice
                        nc.tensor.matmul(prr[:], lhsT=lr, rhs=Ac[qk][:], start=st, stop=False)
                        nc.tensor.matmul(prr[:], lhsT=li, rhs=(An if plus else As)[qk][:], start=False, stop=sp)
                        # Ri = Tr.(+/-Asin) + Ti.Acos
                        nc.tensor.matmul(pri[:], lhsT=lr, rhs=(As if plus else An)[qk][:], start=st, stop=False)
                        nc.tensor.matmul(pri[:], lhsT=li, rhs=Ac[qk][:], start=False, stop=sp)
                    rr = ivp.tile([mw, N], f32r, name="rr", tag=f"rr{js.start}")
                    ri = ivp.tile([mw, N], f32r, name="ri", tag=f"ri{js.start}")
                    nc.scalar.copy(rr[:], prr[:])
                    nc.scalar.copy(ri[:], pri[:])
                    Rr.append(rr)
                    Ri.append(ri)
                # j=256 column strip (uses '+' signs)
                p6r = psN.tile([1, N], f32, name="p6r", tag="psn")
                p6i = psN.tile([1, N], f32, name="p6i", tag="psn")
                for qk in range(3):
                    st, sp = (qk == 0), (qk == 2)
                    if qk < 2:
                        lr, li = Tr[qk][:, 256:257], Ti[qk][:, 256:257]
                    else:
                        lr, li = tnr[:, 256:257], tni[:, 256:257]
                    nc.tensor.matmul(p6r[:], lhsT=lr, rhs=Ac[qk][:], start=st, stop=False)
                    nc.tensor.matmul(p6r[:], lhsT=li, rhs=An[qk][:], start=False, stop=sp)
                    nc.tensor.matmul(p6i[:], lhsT=lr, rhs=As[qk][:], start=st, stop=False)
                    nc.tensor.matmul(p6i[:], lhsT=li, rhs=Ac[qk][:], start=False, stop=sp)
                r6r = ivp.tile([1, N], f32r, name="r6r", tag="r6r")
                r6i = ivp.tile([1, N], f32r, name="r6i", tag="r6i")
                nc.scalar.copy(r6r[:], p6r[:])
                nc.scalar.copy(r6i[:], p6i[:])

                # inv stage 2: y[r, n] = sum_j Bcos[j,r].Rr[j,n] + Bsin[j,r].Ri[j,n]
                for rc in range(4):
                    rs = slice(rc * 128, (rc + 1) * 128)
                    py = psB.tile([128, N], f32, name="py", tag="ps2")
                    kws = [128, 128, 127, 128]
                    for ji in range(5):
                        st = (ji == 0)
                        if ji < 4:
                            kw = kws[ji]
                            nc.tensor.matmul(py[:], lhsT=Bc[ji][0:kw, rs], rhs=Rr[ji][:], start=st, stop=False)
                            nc.tensor.matmul(py[:], lhsT=Bs[ji][0:kw, rs], rhs=Ri[ji][:], start=False, stop=False)
                        else:
                            nc.tensor.matmul(py[:], lhsT=Bc[4][0:1, rs], rhs=r6r[:], start=False, stop=False)
                            nc.tensor.matmul(py[:], lhsT=Bs[4][0:1, rs], rhs=r6i[:], start=False, stop=True)
                    yt = tp2.tile([128, N], f32, name="yt", tag="yt")
                    nc.scalar.copy(yt[:], py[:])
                    nc.scalar.dma_start(y_out[rs, :], yt[:])

            # image stream with one-ahead prefetch (load + pre-adds) so the
            # next image's DVE prep lands before the current image's cmul
            stream = []
            for _rep in range(repeat):
                stream += [kd_in[c] for c in range(CL)]
                stream += [xs_in[b * CL + c]
                           for b in range(NB) for c in range(CL)]
            pstate = {"i": 0, "ud": prep_image(stream[0])}

            def next_image():
                i = pstate["i"]
                ud = pstate["ud"]
                if i + 1 < len(stream):
                    pstate["ud"] = prep_image(stream[i + 1])
                pstate["i"] = i + 1
                return ud

            prev_rs_out = None
            for _rep in range(repeat):
                rdt = f32 if RS_F32 else bf16
                rs_in = dp.tile([NCORES, PB], rdt, name="rs_in", tag="rs_in")
                rs_out = dp.tile([PB], rdt, name="rs_out", tag="rs_out")
                # ---- phase 1: kernel channels ----
                kfr = [[kp.tile([128, W], bf16, name=f"kfr{c}_{qc}", tag=f"kfr{c}_{qc}")
                        for qc in range(2)] for c in range(CL)]
                kfi = [[kp.tile([128, W], bf16, name=f"kfi{c}_{qc}", tag=f"kfi{c}_{qc}")
                        for qc in range(2)] for c in range(CL)]
                nyqk = nyp.tile([1, CL * 2 * NE], f32, name="nyqk", tag="nyqrow")
                for c in range(CL):
                    forward_image(next_image(), "k", c, (kfr[c], kfi[c]), nyqk)
                kfn = kp.tile([CL, 2 * NE], f32, name="kfn", tag="kfn")
                nydk = dp.tile([CL * 2 * NE], f32, name="nydk", tag="nyd")
                nc.gpsimd.dma_start(
                    nydk[:].rearrange("(p f) -> p f", p=1), nyqk[0:1, :])
                nc.gpsimd.dma_start(
                    kfn[:], nydk[:].rearrange("(c f) -> c f", c=CL))

                # ---- phase 2: batches (prev iteration's inverse after b1) ----
                for b in range(NB):
                    acc = ap.tile([128, 4 * W], bf16, name="acc", tag="acc")
                    nyqx = nyp.tile([1, CL * 2 * NE], f32, name="nyqx", tag="nyqrow")
                    for c in range(CL):
                        forward_image(next_image(), "x", c,
                                      (kfr[c], kfi[c]), nyqx, acc=acc)
                    xn = ap.tile([CL, 2 * NE], f32, name="xn", tag="xn")
                    nydx = dp.tile([CL * 2 * NE], f32, name="nydx", tag="nyd")
                    nc.gpsimd.dma_start(
                        nydx[:].rearrange("(p f) -> p f", p=1), nyqx[0:1, :])
                    nc.gpsimd.dma_start(
                        xn[:], nydx[:].rearrange("(c f) -> c f", c=CL))
                    # nyq pointwise (per-channel rows) + channel reduce via K=4 matmul
                    xnr, xni = xn[:, 0:NE], xn[:, NE:2 * NE]
                    kfnr, kfni = kfn[:, 0:NE], kfn[:, NE:2 * NE]
                    t1 = tp2.tile([CL, NE], f32, name="nt1", tag="nt")
                    t2 = tp2.tile([CL, NE], f32, name="nt2", tag="nt")
                    un = tp2.tile([CL, NE], f32r, name="un", tag="un")
                    vn = tp2.tile([CL, NE], f32r, name="vn", tag="un")
                    nc.vector.tensor_tensor(t1[:], xnr, kfnr, op=mult)
                    nc.vector.tensor_tensor(t2[:], xni, kfni, op=mult)
                    nc.vector.tensor_tensor(un[:], t1[:], t2[:], op=sub)
                    nc.vector.tensor_tensor(t1[:], xnr, kfni, op=mult)
                    nc.vector.tensor_tensor(t2[:], xni, kfnr, op=mult)
                    nc.vector.tensor_tensor(vn[:], t1[:], t2[:], op=add)
                    pnr = psN.tile([1, NE], f32, name="pnr", tag="psn")
                    pni = psN.tile([1, NE], f32, name="pni", tag="psn")
                    nc.tensor.matmul(pnr[:], lhsT=o4[0][:], rhs=un[:], start=True, stop=True)
                    nc.tensor.matmul(pni[:], lhsT=o4[0][:], rhs=vn[:], start=True, stop=True)
                    sn2 = tp2.tile([1, 2 * NE], f32 if RS_F32 else bf16,
                                   name="sn2", tag="sn")
                    nc.scalar.copy(sn2[:, 0:NE], pnr[:])
                    nc.scalar.copy(sn2[:, NE:2 * NE], pni[:])
                    # ship batch partial to rs_in row b (single big DMA + nyq)
                    row = rs_in[b]
                    if RS_F32:
                        accf = nyp.tile([128, 4 * W], f32, name="accf", tag="accf")
                        nc.scalar.copy(accf[:], acc[:])
                        src_acc = accf
                    else:
                        src_acc = acc
                    nc.gpsimd.dma_start(
                        row[0:4 * 128 * W].rearrange("(p f) -> p f", p=128),
                        src_acc[:])
                    nyb = 4 * 128 * W
                    nc.gpsimd.dma_start(
                        row[nyb:nyb + 2 * NE].rearrange("(p f) -> p f", p=1),
                        sn2[:])
                    if b == 1 and prev_rs_out is not None:
                        inverse(prev_rs_out)

                # ---- phase 3: reduce-scatter ----
                nc.gpsimd.collective_compute(
                    "ReduceScatter", mybir.AluOpType.add,
                    replica_groups=[list(range(NCORES))],
                    ins=[rs_in[:].opt()], outs=[rs_out[:].opt()],
                )
                prev_rs_out = rs_out

            # drain: last iteration's inverse
            inverse(prev_rs_out)

    nc.compile()
    return nc


def _get_nc(repeat=1):
    key = f"nc{repeat}"
    if key not in _CACHE:
        _CACHE[key] = _build_nc(repeat)
    return _CACHE[key]


def _get_runner(repeat=1):
    """Build (once) and cache a jitted SPMD executable for the bass program.

    Replicates concourse.bass2jax.run_bass_via_pjrt, but keeps the jitted
    callable alive across kernel() invocations so repeated calls reuse the
    compiled NEFF instead of re-tracing + re-compiling every time.
    """
    key = f"runner{repeat}"
    if key in _CACHE:
        return _CACHE[key]
    import jax
    from jax.sharding import Mesh, PartitionSpec
    from jax.experimental.shard_map import shard_map
    import concourse.bass2jax as b2j
    import concourse.mybir as mybir

    b2j.install_neuronx_cc_hook()
    nc = _get_nc(repeat)
    assert nc.dbg_addr is None
    partition_name = (nc.partition_id_tensor.name
                      if nc.partition_id_tensor else None)
    in_names, out_names, out_avals, zero_shapes = [], [], [], []
    for alloc in nc.m.functions[0].allocations:
        if not isinstance(alloc, mybir.MemoryLocationSet):
            continue
        name = alloc.memorylocations[0].name
        if alloc.kind == "ExternalInput":
            if name != partition_name:
                in_names.append(name)
        elif alloc.kind == "ExternalOutput":
            shape = tuple(alloc.tensor_shape)
            dtype = mybir.dt.np(alloc.dtype)
            out_names.append(name)
            out_avals.append(jax.core.ShapedArray(shape, dtype))
            zero_shapes.append((shape, dtype))
    n_params = len(in_names)
    n_outs = len(out_avals)
    all_in_names = list(in_names) + list(out_names)
    if partition_name is not None:
        all_in_names.append(partition_name)
    donate = tuple(range(n_params, n_params + n_outs))

    def _body(*args):
        operands = list(args)
        if partition_name is not None:
            operands.append(b2j.partition_id_tensor())
        outs = b2j._bass_exec_p.bind(
            *operands,
            out_avals=tuple(out_avals),
            in_names=tuple(all_in_names),
            out_names=tuple(out_names),
            lowering_input_output_aliases=(),
            sim_require_finite=True,
            sim_require_nnan=True,
            nc=nc,
        )
        return tuple(outs)

    devices = jax.devices()[:NCORES]
    mesh = Mesh(np.asarray(devices), ("core",))
    in_specs = (PartitionSpec("core"),) * (n_params + n_outs)
    out_specs = (PartitionSpec("core"),) * n_outs
    sharded = jax.jit(
        shard_map(_body, mesh=mesh, in_specs=in_specs,
                  out_specs=out_specs, check_rep=False),
        donate_argnums=donate, keep_unused=True,
    )
    runner = (sharded, in_names, out_names, out_avals, zero_shapes)
    _CACHE[key] = runner
    return runner


def _run_spmd(in_maps, repeat=1):
    sharded, in_names, out_names, out_avals, zero_shapes = _get_runner(repeat)
    concat_in = [
        np.concatenate([np.asarray(in_maps[c][name]) for c in range(NCORES)],
                       axis=0)
        for name in in_names
    ]
    concat_zeros = [
        np.zeros((NCORES * sh[0], *sh[1:]), dt) for (sh, dt) in zero_shapes
    ]
    out_arrs = sharded(*concat_in, *concat_zeros)
    return [
        {name: np.asarray(out_arrs[i]).reshape(NCORES, *out_avals[i].shape)[c]
         for i, name in enumerate(out_names)}
        for c in range(NCORES)
    ]


def build_in_maps(x, kernel, variable_kernel, loc_idx):
    x = np.asarray(x)
    vk = np.asarray(variable_kernel)
    idx = np.asarray(loc_idx)
    # host scatter of relu'd values into the dense PSF (data movement only)
    kflat = np.asarray(kernel)[0].reshape(-1).copy()
    kflat[idx] = np.maximum(vk, 0.0).astype(np.float32)
    kd_all = kflat.reshape(NC_TOT, N, N)

    C = _consts()
    in_maps = []
    for core in range(NCORES):
        c0 = core * CL
        m = {nm: C[nm] for nm in CONST_SHAPES}
        m["xs"] = np.ascontiguousarray(
            x[:, c0:c0 + CL].reshape(NB * CL, N, N)).astype(np.float32)
        m["kd"] = np.ascontiguousarray(kd_all[c0:c0 + CL]).astype(np.float32)
        in_maps.append(m)
    return in_maps


def kernel(x, kernel, variable_kernel, loc_idx, _repeat=1):
    in_maps = build_in_maps(x, kernel, variable_kernel, loc_idx)
    res = _run_spmd(in_maps, _repeat)
    out = np.stack([res[b]["y"] for b in range(NB)], axis=0)
    return out.astype(np.float32)
